# revision 27
# baseline (speedup 1.0000x reference)
"""DirVRNN loss kernel for 8 Trainium2 NeuronCores (Bass/Tile).

Strategy: pure data parallel. Batch 1024 -> 8 cores x 128. Per core the whole
recurrence (32 windows x 16 steps + per-window 16-step LSTM decoder) runs
on-chip with feature-major activations [feat, batch=128], bf16 matmuls with
fp32 PSUM accumulation. Each core emits partial sums; the host combines them
into the scalar loss.
"""

import os
import sys

import numpy as np

for _p in ("/opt/trn_rl_repo",):
    if _p not in sys.path and os.path.isdir(_p):
        sys.path.insert(0, _p)

import ml_dtypes

BF16 = ml_dtypes.bfloat16

# ---------------- problem dims (hardcoded per spec) ----------------
B, S, D = 1024, 512, 64
L, KCL, NCLS = 64, 16, 4
NH = 256
WWIN = 16
H2 = 2 * L            # 128
NCORES = 8
BL = B // NCORES      # 128 batch per core
NWIN_FULL = S // WWIN  # 32
NT = WWIN * BL        # 2048 columns per window
LOG2PI = float(np.log(2.0 * np.pi))

F32 = np.float32

_CACHE = {}
LAST_RESULTS = None  # BassKernelResults of last run (for test.py)


def _bf(a):
    return np.ascontiguousarray(np.asarray(a, dtype=F32).astype(BF16))


def _f32(a):
    return np.ascontiguousarray(np.asarray(a, dtype=F32))


# ---------------- packed-constant layout (shared host/device) ----------------
# name -> (partitions, cols, dtype_tag)  dtype_tag in {"bf", "f32"}
def _const_specs():
    sp = {}
    for mlp in ("x", "p", "e", "z", "c", "d"):
        sp[f"w2{mlp}_0"] = (128, 256, "bf")
        sp[f"w2{mlp}_1"] = (128, 256, "bf")
        sp[f"b2{mlp}"] = (128, 2, "f32")
    sp.update({
        "w1x_aug": (65, 256, "bf"),
        "w3x_0": (128, 65, "bf"), "w3x_1": (128, 65, "bf"),
        "b3x_aug": (65, 1, "f32"),
        "w1p_aug": (65, 256, "bf"),
        "w3p_0": (128, 16, "bf"), "w3p_1": (128, 16, "bf"),
        "b3p_r": (1, 16, "bf"),
        "w1e_x": (65, 256, "bf"), "w1e_h": (64, 256, "bf"),
        "w3e_0": (128, 16, "bf"), "w3e_1": (128, 16, "bf"),
        "b3e_r": (1, 16, "bf"),
        "w1z_aug": (65, 256, "bf"),
        "w3z_0": (128, 65, "bf"), "w3z_1": (128, 65, "bf"),
        "b3z_aug": (65, 1, "f32"),
        "w1c_h": (64, 256, "bf"), "w1c_x": (65, 256, "bf"),
        "w1c_u": (64, 256, "bf"),
        "w3c_0": (128, 65, "bf"), "w3c_1": (128, 65, "bf"),
        "b3c_aug": (65, 1, "f32"),
        "w1d_aug": (65, 256, "bf"),
        "w3d_0": (128, 4, "bf"), "w3d_1": (128, 4, "bf"),
        "b3d_r": (1, 4, "bf"),
        "wih_u": (65, 512, "bf"), "wih_h": (64, 512, "bf"),
        "whh0_u": (64, 512, "bf"), "whh0_h": (64, 512, "bf"),
        "whh_full": (128, 512, "bf"),
        "wout_mu": (128, 64, "bf"), "wout_lv": (128, 64, "bf"),
        "nlv2": (64, 1, "f32"), "bmu": (64, 1, "f32"),
        "cma": (16, 65, "bf"),
        "i128": (128, 128, "bf"),
        "identf": (128, 128, "f32"),
        "ones1": (1, 128, "bf"),
        "u0b": (65, BL, "bf"), "h0b": (65, BL, "bf"),
        "yin": (BL, NCLS, "f32"),
    })
    return sp


def _pack_layout():
    """Deterministic column layout of the two constant packs."""
    specs = _const_specs()
    lay = {}
    off = {"bf": 0, "f32": 0}
    for name in sorted(specs):
        p, c, tag = specs[name]
        lay[name] = (p, c, tag, off[tag])
        off[tag] += (c + 7) // 8 * 8
    return lay, off["bf"], off["f32"]


# =====================================================================
# device program
# =====================================================================

def _build(nwin, parts=("phix", "dec", "inner", "ep")):
    import concourse.bass as bass
    import concourse.tile as tile
    from concourse import bacc, mybir

    AF = mybir.ActivationFunctionType
    ALU = mybir.AluOpType
    AX = mybir.AxisListType
    f32 = mybir.dt.float32
    bf16 = mybir.dt.bfloat16

    nc = bacc.Bacc("TRN2", target_bir_lowering=False, detect_race_conditions=False)

    # ---------------- dram inputs ----------------
    lay, ncol_bf, ncol_f32 = _pack_layout()
    wbf_d = nc.dram_tensor("wbf", [128, ncol_bf], bf16, kind="ExternalInput")
    wf_d = nc.dram_tensor("wf", [128, ncol_f32], f32, kind="ExternalInput")
    xf_d = nc.dram_tensor("xf", [nwin, 64, NT], f32, kind="ExternalInput")
    xb_d = nc.dram_tensor("xbf", [nwin, 65, NT], bf16, kind="ExternalInput")
    out_d = nc.dram_tensor("out", [3, 128], f32, kind="ExternalOutput")

    with tile.TileContext(nc) as tc:
        from contextlib import ExitStack

        with ExitStack() as ctx:
            wp = ctx.enter_context(tc.tile_pool(name="wp", bufs=1))
            xp = ctx.enter_context(tc.tile_pool(name="xp", bufs=2))
            ap = ctx.enter_context(tc.tile_pool(name="ap", bufs=4))   # per-step acts
            cp = ctx.enter_context(tc.tile_pool(name="cp", bufs=4))   # carries
            sp = ctx.enter_context(tc.tile_pool(name="sp", bufs=1))   # persistent stash
            psA = ctx.enter_context(tc.tile_pool(name="psA", bufs=4, space="PSUM"))
            psB = ctx.enter_context(tc.tile_pool(name="psB", bufs=1, space="PSUM"))
            psC = ctx.enter_context(tc.tile_pool(name="psC", bufs=1, space="PSUM"))
            psD = ctx.enter_context(tc.tile_pool(name="psD", bufs=1, space="PSUM"))
            psE = ctx.enter_context(tc.tile_pool(name="psE", bufs=1, space="PSUM"))

            wbf_t = wp.tile([128, ncol_bf], bf16, name="wbf_t", tag="wbf_t")
            nc.sync.dma_start(out=wbf_t, in_=wbf_d[:, :])
            wf_t = wp.tile([128, ncol_f32], f32, name="wf_t", tag="wf_t")
            nc.sync.dma_start(out=wf_t, in_=wf_d[:, :])
            W = {}
            for name, (p, c, tag, off) in lay.items():
                src = wbf_t if tag == "bf" else wf_t
                W[name] = src[0:p, off:off + c]
            ysb = W["yin"]

            # warm-up: make every engine observe each pack's DMA semaphore
            # before any real op, so no compute op carries >1 DMA wait.
            wu = wp.tile([1, 16], f32, name="wu", tag="wu")
            nc.vector.tensor_copy(wu, wf_t[0:1, 0:16])
            nc.vector.tensor_copy(wu, wbf_t[0:1, 0:16])
            nc.scalar.activation(wu, wf_t[0:1, 0:16], AF.Copy)
            nc.scalar.activation(wu, wbf_t[0:1, 0:16], AF.Copy)

            # persistent accumulators / stashes
            sestash = sp.tile([128, S], f32, name="sestash", tag="sestash")
            spstash = sp.tile([128, S], f32, name="spstash", tag="spstash")
            klacc = sp.tile([128, 1], f32, name="klacc", tag="klacc")
            acc1 = sp.tile([64, 1], f32, name="acc1", tag="acc1")
            acc2 = sp.tile([64, 1], f32, name="acc2", tag="acc2")
            nc.vector.memset(klacc, 0.0)
            nc.vector.memset(acc1, 0.0)
            nc.vector.memset(acc2, 0.0)

            # initial carries (read-only slices of the constant pack)
            u_ap = W["u0b"]
            h_ap = W["h0b"]

            mmop = nc.tensor.matmul

            def relu_evac(eng, out, ps, bias=None):
                if eng == "s":
                    nc.scalar.activation(out, ps, AF.Relu,
                                         bias=bias if bias is not None else 0.0)
                else:
                    if bias is None:
                        nc.vector.tensor_scalar(out, ps, 0.0, None, ALU.max)
                    else:
                        nc.vector.tensor_scalar(out, ps, bias, 0.0, ALU.add, ALU.max)

            def copy_evac(eng, out, ps):
                if eng == "s":
                    nc.scalar.activation(out, ps, AF.Copy)
                else:
                    nc.vector.tensor_copy(out, ps)

            def relu_evac2(out, ps, ncols, bias0=None, bias1=None):
                """Split a [P, ncols] relu-evac across ACT and DVE."""
                h = ncols // 2
                if bias0 is None:
                    nc.scalar.activation(out[:, 0:h], ps[:, 0:h], AF.Relu)
                    nc.vector.tensor_scalar(out[:, h:ncols], ps[:, h:ncols],
                                            0.0, None, ALU.max)
                else:
                    nc.scalar.activation(out[:, 0:h], ps[:, 0:h], AF.Relu,
                                         bias=bias0)
                    nc.vector.tensor_scalar(out[:, h:ncols], ps[:, h:ncols],
                                            bias1, 0.0, ALU.add, ALU.max)

            # engine assignment per op site ("s"=ScalarE/ACT, "v"=VectorE/DVE)
            E = {
                "enc1": "s", "enc2": "v", "prior1": "s", "prior2": "v",
                "phiz1": "v", "phiz2": "v", "phiz3": "s",
                "cell1": "s", "cell2": "s", "cell3": "s",
                "px1": "s", "px2": "v", "px3": "s",
                "piT": "v", "zev": "s", "cwev": "s", "lpev": "s",
            }

            ts = bass.ts

            def dma_window(w):
                xf_t = xp.tile([64, NT], f32, name="xf_t", tag="xf_t")
                nc.sync.dma_start(out=xf_t, in_=xf_d[w, :, :])
                xbf_t = xp.tile([65, NT], bf16, name="xbf_t", tag="xbf_t")
                nc.sync.dma_start(out=xbf_t, in_=xb_d[w, :, :])
                xb_t = xp.tile([64, NT], f32, name="xb_t", tag="xb_t")
                nc.vector.tensor_scalar(xb_t, xf_t, W["bmu"][:, 0:1], None,
                                        ALU.subtract)
                phx_t = xp.tile([65, NT], bf16, name="phx_t", tag="phx_t")
                return xbf_t, xb_t, phx_t

            def phix_chunk(nch, xbf_t, phx_t):
                cs = slice(nch * 512, (nch + 1) * 512)
                p1a = psB.tile([128, 512], f32, name="p1a", tag="psB")
                mmop(p1a, W["w1x_aug"][:, 0:128], xbf_t[:, cs],
                     start=True, stop=True)
                p1b = psB.tile([128, 512], f32, name="p1b", tag="psB")
                mmop(p1b, W["w1x_aug"][:, 128:256], xbf_t[:, cs],
                     start=True, stop=True)
                ph1 = xp.tile([128, 1024], bf16, name="ph1", tag="ph1")
                relu_evac(E["px1"], ph1[:, 0:512], p1a)
                relu_evac(E["px1"], ph1[:, 512:1024], p1b)
                p2a = psB.tile([128, 512], f32, name="p2a", tag="psB")
                mmop(p2a, W["w2x_0"][:, 0:128], ph1[:, 0:512],
                     start=True, stop=False)
                mmop(p2a, W["w2x_1"][:, 0:128], ph1[:, 512:1024],
                     start=False, stop=True)
                p2b = psB.tile([128, 512], f32, name="p2b", tag="psB")
                mmop(p2b, W["w2x_0"][:, 128:256], ph1[:, 0:512],
                     start=True, stop=False)
                mmop(p2b, W["w2x_1"][:, 128:256], ph1[:, 512:1024],
                     start=False, stop=True)
                ph2 = xp.tile([128, 1024], bf16, name="ph2", tag="ph2")
                if E["px2"] == "s":
                    nc.scalar.activation(ph2[:, 0:512], p2a, AF.Relu,
                                         bias=W["b2x"][:, 0:1])
                    nc.scalar.activation(ph2[:, 512:1024], p2b, AF.Relu,
                                         bias=W["b2x"][:, 1:2])
                else:
                    nc.vector.tensor_scalar(ph2[:, 0:512], p2a,
                                            W["b2x"][:, 0:1], 0.0,
                                            ALU.add, ALU.max)
                    nc.vector.tensor_scalar(ph2[:, 512:1024], p2b,
                                            W["b2x"][:, 1:2], 0.0,
                                            ALU.add, ALU.max)
                p3 = psB.tile([65, 512], f32, name="p3", tag="psB")
                mmop(p3, W["w3x_0"], ph2[:, 0:512], start=True, stop=False)
                mmop(p3, W["w3x_1"], ph2[:, 512:1024], start=False, stop=True)
                relu_evac(E["px3"], phx_t[:, cs], p3, bias=W["b3x_aug"][:, 0:1])

            # prologue: window 0 x + phix
            xbf_cur, xb_cur, phx_cur = dma_window(0)
            for nch in range(4):
                phix_chunk(nch, xbf_cur, phx_cur)

            for w in range(nwin):
                if w + 1 < nwin:
                    xbf_nxt, xb_nxt, phx_nxt = dma_window(w + 1)
                phx = phx_cur
                xb_t = xb_cur
                u_prev, h_prev = u_ap, h_ap

                # ---- decoder: cw = ctx @ Wih + bl (hoisted once per window) ----
                pcw = psA.tile([128, 512], f32, name="pcw", tag="psA")
                for g in range(4):
                    gs = slice(g * 128, (g + 1) * 128)
                    mmop(pcw[:, gs], W["wih_u"][:, gs], u_prev[0:65, :],
                         start=True, stop=False)
                    mmop(pcw[:, gs], W["wih_h"][:, gs], h_prev[0:64, :],
                         start=False, stop=True)
                cw = cp.tile([128, 512], bf16, name="cw", tag="cw")
                copy_evac(E["cwev"], cw, pcw)

                # ---- logits psum stash for the window ----
                pl = psD.tile([128, 512], f32, name="pl", tag="psD")
                le = pl[:, 0:256]
                lp = pl[:, 256:512]
                pist = xp.tile([128, 256], f32, name="pist", tag="pist")

                hd_prev = None
                cd_prev = None
                hd4 = None

                # ---- inner recurrence, decoder + next-window phix interleaved
                for t in range(WWIN):
                    tgcol = w * WWIN + t
                    h_cur = h_ap
                    # enc L1
                    pe1 = psA.tile([128, 256], f32, name="pe1", tag="psA")
                    for m in range(2):
                        ms = slice(m * 128, (m + 1) * 128)
                        mmop(pe1[:, ms], W["w1e_x"][:, ms], phx[:, ts(t, 128)],
                             start=True, stop=False)
                        mmop(pe1[:, ms], W["w1e_h"][:, ms], h_ap[0:64, :],
                             start=False, stop=True)
                    h1e = ap.tile([128, 256], bf16, name="h1e", tag="h1e")
                    relu_evac(E["enc1"], h1e, pe1)
                    # enc L2
                    pe2 = psA.tile([128, 256], f32, name="pe2", tag="psA")
                    for m in range(2):
                        ms = slice(m * 128, (m + 1) * 128)
                        mmop(pe2[:, ms], W["w2e_0"][:, ms], h1e[:, 0:128],
                             start=True, stop=False)
                        mmop(pe2[:, ms], W["w2e_1"][:, ms], h1e[:, 128:256],
                             start=False, stop=True)
                    h2e = ap.tile([128, 256], bf16, name="h2e", tag="h2e")
                    for m in range(2):
                        ms = slice(m * 128, (m + 1) * 128)
                        relu_evac(E["enc2"], h2e[:, ms], pe2[:, ms],
                                  bias=W["b2e"][:, m:m + 1])
                    # enc logits (batch-major)
                    lesl = le[:, t * 16:(t + 1) * 16]
                    mmop(lesl, W["ones1"], W["b3e_r"], start=True, stop=False)
                    mmop(lesl, h2e[:, 0:128], W["w3e_0"], start=False,
                         stop=False)
                    mmop(lesl, h2e[:, 128:256], W["w3e_1"], start=False,
                         stop=True)
                    # softmax -> pi, z
                    ep = ap.tile([128, 16], f32, name="ep", tag="ep")
                    nc.scalar.activation(ep, lesl, AF.Exp)
                    nc.vector.reduce_sum(sestash[:, tgcol:tgcol + 1], ep, AX.X)
                    rcp = ap.tile([128, 1], f32, name="rcp", tag="rcp")
                    nc.vector.reciprocal(rcp, sestash[:, tgcol:tgcol + 1])
                    pisl = pist[:, t * 16:(t + 1) * 16]
                    nc.vector.tensor_scalar(pisl, ep, rcp, None, ALU.mult)
                    pz = psE.tile([128, 256], f32, name="pz", tag="psE")
                    nc.tensor.transpose(pz[0:16, 0:128], pisl, W["identf"])
                    piT = ap.tile([16, 128], bf16, name="piT", tag="piT")
                    copy_evac(E["piT"], piT, pz[0:16, 0:128])
                    mmop(pz[0:65, 128:256], W["cma"], piT, start=True,
                         stop=True)
                    z_ap = cp.tile([65, BL], bf16, name="z_ap", tag="z")
                    copy_evac(E["zev"], z_ap, pz[0:65, 128:256])
                    # phiz
                    pz1 = psA.tile([128, 256], f32, name="pz1", tag="psA")
                    for m in range(2):
                        ms = slice(m * 128, (m + 1) * 128)
                        mmop(pz1[:, ms], W["w1z_aug"][:, ms], z_ap[0:65, :],
                             start=True, stop=True)
                    h1z = ap.tile([128, 256], bf16, name="h1z", tag="h1z")
                    relu_evac(E["phiz1"], h1z, pz1)
                    pz2 = psA.tile([128, 256], f32, name="pz2", tag="psA")
                    for m in range(2):
                        ms = slice(m * 128, (m + 1) * 128)
                        mmop(pz2[:, ms], W["w2z_0"][:, ms], h1z[:, 0:128],
                             start=True, stop=False)
                        mmop(pz2[:, ms], W["w2z_1"][:, ms], h1z[:, 128:256],
                             start=False, stop=True)
                    h2z = ap.tile([128, 256], bf16, name="h2z", tag="h2z")
                    for m in range(2):
                        ms = slice(m * 128, (m + 1) * 128)
                        relu_evac(E["phiz2"], h2z[:, ms], pz2[:, ms],
                                  bias=W["b2z"][:, m:m + 1])
                    pz3 = psA.tile([65, 128], f32, name="pz3", tag="psA")
                    mmop(pz3, W["w3z_0"], h2z[:, 0:128], start=True, stop=False)
                    mmop(pz3, W["w3z_1"], h2z[:, 128:256], start=False,
                         stop=True)
                    u_new = cp.tile([65, BL], bf16, name="u_new", tag="u")
                    relu_evac(E["phiz3"], u_new, pz3, bias=W["b3z_aug"][:, 0:1])
                    # cell
                    pc1 = psA.tile([128, 256], f32, name="pc1", tag="psA")
                    for m in range(2):
                        ms = slice(m * 128, (m + 1) * 128)
                        mmop(pc1[:, ms], W["w1c_x"][:, ms], phx[:, ts(t, 128)],
                             start=True, stop=False)
                        mmop(pc1[:, ms], W["w1c_h"][:, ms], h_ap[0:64, :],
                             start=False, stop=False)
                        mmop(pc1[:, ms], W["w1c_u"][:, ms], u_new[0:64, :],
                             start=False, stop=True)
                    h1c = ap.tile([128, 256], bf16, name="h1c", tag="h1c")
                    relu_evac(E["cell1"], h1c, pc1)
                    pc2 = psA.tile([128, 256], f32, name="pc2", tag="psA")
                    for m in range(2):
                        ms = slice(m * 128, (m + 1) * 128)
                        mmop(pc2[:, ms], W["w2c_0"][:, ms], h1c[:, 0:128],
                             start=True, stop=False)
                        mmop(pc2[:, ms], W["w2c_1"][:, ms], h1c[:, 128:256],
                             start=False, stop=True)
                    h2c = ap.tile([128, 256], bf16, name="h2c", tag="h2c")
                    for m in range(2):
                        ms = slice(m * 128, (m + 1) * 128)
                        relu_evac(E["cell2"], h2c[:, ms], pc2[:, ms],
                                  bias=W["b2c"][:, m:m + 1])
                    pc3 = psA.tile([65, 128], f32, name="pc3", tag="psA")
                    mmop(pc3, W["w3c_0"], h2c[:, 0:128], start=True, stop=False)
                    mmop(pc3, W["w3c_1"], h2c[:, 128:256], start=False,
                         stop=True)
                    h_new = cp.tile([65, BL], bf16, name="h_new", tag="h")
                    relu_evac(E["cell3"], h_new, pc3, bias=W["b3c_aug"][:, 0:1])

                    u_ap = u_new
                    h_ap = h_new

                    # prior MLP (off-path: emitted late so it fills bubbles)
                    pp1 = psA.tile([128, 256], f32, name="pp1", tag="psA")
                    for m in range(2):
                        ms = slice(m * 128, (m + 1) * 128)
                        mmop(pp1[:, ms], W["w1p_aug"][:, ms], h_cur[0:65, :],
                             start=True, stop=True)
                    h1p = ap.tile([128, 256], bf16, name="h1p", tag="h1p")
                    relu_evac(E["prior1"], h1p, pp1)
                    # prior L2
                    pp2 = psA.tile([128, 256], f32, name="pp2", tag="psA")
                    for m in range(2):
                        ms = slice(m * 128, (m + 1) * 128)
                        mmop(pp2[:, ms], W["w2p_0"][:, ms], h1p[:, 0:128],
                             start=True, stop=False)
                        mmop(pp2[:, ms], W["w2p_1"][:, ms], h1p[:, 128:256],
                             start=False, stop=True)
                    h2p = ap.tile([128, 256], bf16, name="h2p", tag="h2p")
                    for m in range(2):
                        ms = slice(m * 128, (m + 1) * 128)
                        relu_evac(E["prior2"], h2p[:, ms], pp2[:, ms],
                                  bias=W["b2p"][:, m:m + 1])
                    lpsl = lp[:, t * 16:(t + 1) * 16]
                    mmop(lpsl, W["ones1"], W["b3p_r"], start=True, stop=False)
                    mmop(lpsl, h2p[:, 0:128], W["w3p_0"], start=False,
                         stop=False)
                    mmop(lpsl, h2p[:, 128:256], W["w3p_1"], start=False,
                         stop=True)

                    # ---- decoder LSTM step j = t (independent chain) ----
                    j = t
                    if j % 4 == 0:
                        hd4 = cp.tile([128, 512], bf16, name="hd4", tag="hd4",
                                      bufs=2)
                    pg = psC.tile([128, 512], f32, name="pg", tag="psC")
                    for g in range(4):
                        gs = slice(g * 128, (g + 1) * 128)
                        if j == 0:
                            mmop(pg[:, gs], W["whh0_u"][:, gs], u_prev[0:64, :],
                                 start=True, stop=False)
                            mmop(pg[:, gs], W["whh0_h"][:, gs], h_prev[0:64, :],
                                 start=False, stop=False)
                        else:
                            mmop(pg[:, gs], W["whh_full"][:, gs], hd_prev,
                                 start=True, stop=False)
                    mmop(pg, W["i128"], cw, start=False, stop=True,
                         skip_group_check=True)
                    # g-gate weights are pre-doubled: one tanh(psum/2) does
                    # both the 3 sigmoids' halves and the raw g tanh.
                    tall = ap.tile([128, 512], bf16, name="tall", tag="tall",
                                   bufs=2)
                    nc.scalar.activation(tall, pg, AF.Tanh, scale=0.5)
                    tg = tall[:, 384:512]
                    # sigmoid(g) = 0.5*tanh(g/2) + 0.5
                    sig = ap.tile([128, 384], bf16, name="sig", tag="sig",
                                  bufs=2)
                    nc.vector.tensor_scalar(sig, tall[:, 0:384], 0.5, 0.5,
                                            ALU.mult, ALU.add)
                    t1 = ap.tile([128, 128], bf16, name="t1", tag="t1", bufs=2)
                    nc.vector.tensor_tensor(t1, sig[:, 0:128], tg, ALU.mult)
                    if j == 0:
                        cd = t1
                    else:
                        t2 = ap.tile([128, 128], bf16, name="t2", tag="t2",
                                     bufs=2)
                        nc.vector.tensor_tensor(t2, sig[:, 128:256], cd_prev,
                                                ALU.mult)
                        cd = ap.tile([128, 128], bf16, name="cd", tag="cd",
                                     bufs=2)
                        nc.vector.tensor_tensor(cd, t1, t2, ALU.add)
                    cd_prev = cd
                    tcd = ap.tile([128, 128], bf16, name="tcd", tag="tcd",
                                  bufs=2)
                    nc.scalar.activation(tcd, cd, AF.Tanh)
                    hs = slice((j % 4) * 128, (j % 4 + 1) * 128)
                    nc.vector.tensor_tensor(hd4[:, hs], sig[:, 256:384], tcd,
                                            ALU.mult)
                    hd_prev = hd4[:, hs]

                    if j % 4 == 3:
                        g4 = j // 4
                        pmu = psB.tile([64, 512], f32, name="pmu", tag="psB")
                        mmop(pmu, W["wout_mu"], hd4, start=True, stop=True)
                        plv = psB.tile([64, 512], f32, name="plv", tag="psB")
                        mmop(plv, W["wout_lv"], hd4, start=True, stop=True)
                        e2 = ap.tile([64, 512], f32, name="e2", tag="e2", bufs=2)
                        nc.scalar.activation(e2, plv, AF.Exp, scale=-0.5,
                                             bias=W["nlv2"][:, 0:1])
                        dt_ = ap.tile([64, 512], f32, name="dt_", tag="dt_",
                                      bufs=2)
                        nc.vector.tensor_tensor(
                            dt_, xb_t[:, g4 * 512:(g4 + 1) * 512], pmu,
                            ALU.subtract)
                        qt = ap.tile([64, 512], f32, name="qt", tag="qt", bufs=2)
                        nc.vector.tensor_tensor(qt, dt_, e2, ALU.mult)
                        jq = ap.tile([64, 512], f32, name="jq", tag="jq", bufs=2)
                        sq = ap.tile([64, 1], f32, name="sq", tag="sq", bufs=2)
                        nc.scalar.activation(jq, qt, AF.Square, accum_out=sq)
                        nc.vector.tensor_tensor(acc1, acc1, sq, ALU.add)
                        lvs = ap.tile([64, 1], f32, name="lvs", tag="lvs",
                                      bufs=2)
                        nc.vector.tensor_reduce(lvs, plv, AX.X, ALU.add)
                        nc.vector.tensor_tensor(acc2, acc2, lvs, ALU.add)

                    # ---- spread next window's phix across this window ----
                    if w + 1 < nwin and t in (2, 6, 10, 14):
                        phix_chunk((t - 2) // 4, xbf_nxt, phx_nxt)

                # ---- KL for the window ----
                lpsb = ap.tile([128, 256], f32, name="lpsb", tag="lpsb", bufs=2)
                copy_evac(E["lpev"], lpsb, lp)
                epp = ap.tile([128, 256], f32, name="epp", tag="epp", bufs=2)
                nc.scalar.activation(epp, lp, AF.Exp)
                nc.vector.tensor_reduce(
                    spstash[:, w * 16:(w + 1) * 16],
                    epp.rearrange("p (t k) -> p t k", k=16), AX.X, ALU.add)
                dif = ap.tile([128, 256], f32, name="dif", tag="dif", bufs=2)
                nc.vector.tensor_tensor(dif, le, lpsb, ALU.subtract)
                jkl = ap.tile([128, 256], f32, name="jkl", tag="jkl", bufs=2)
                nc.vector.tensor_tensor(jkl, pist, dif, ALU.mult)
                rkl = ap.tile([128, 1], f32, name="rkl", tag="rkl", bufs=2)
                nc.vector.tensor_reduce(rkl, jkl, AX.X, ALU.add)
                nc.vector.tensor_tensor(klacc, klacc, rkl, ALU.add)

                if w + 1 < nwin:
                    xbf_cur, xb_cur, phx_cur = xbf_nxt, xb_nxt, phx_nxt

            # ================= epilogue =================
            pd1 = psA.tile([128, 256], f32, name="pd1", tag="psA")
            for m in range(2):
                ms = slice(m * 128, (m + 1) * 128)
                mmop(pd1[:, ms], W["w1d_aug"][:, ms], z_ap[0:65, :],
                     start=True, stop=True)
            h1d = ap.tile([128, 256], bf16, name="h1d", tag="h1d")
            relu_evac("v", h1d, pd1)
            pd2 = psA.tile([128, 256], f32, name="pd2", tag="psA")
            for m in range(2):
                ms = slice(m * 128, (m + 1) * 128)
                mmop(pd2[:, ms], W["w2d_0"][:, ms], h1d[:, 0:128],
                     start=True, stop=False)
                mmop(pd2[:, ms], W["w2d_1"][:, ms], h1d[:, 128:256],
                     start=False, stop=True)
            h2d = ap.tile([128, 256], bf16, name="h2d", tag="h2d")
            for m in range(2):
                ms = slice(m * 128, (m + 1) * 128)
                relu_evac("v", h2d[:, ms], pd2[:, ms], bias=W["b2d"][:, m:m + 1])
            pl4 = psA.tile([128, 4], f32, name="pl4", tag="psA")
            mmop(pl4, W["ones1"], W["b3d_r"], start=True, stop=False)
            mmop(pl4, h2d[:, 0:128], W["w3d_0"], start=False, stop=False)
            mmop(pl4, h2d[:, 128:256], W["w3d_1"], start=False, stop=True)
            ep4 = ap.tile([128, 4], f32, name="ep4", tag="ep4")
            s4 = sp.tile([128, 1], f32, name="s4", tag="s4")
            nc.scalar.activation(ep4, pl4, AF.Exp, accum_out=s4)
            cejunk = ap.tile([128, 4], f32, name="cejunk", tag="cejunk")
            ceacc = sp.tile([128, 1], f32, name="ceacc", tag="ceacc")
            nc.vector.tensor_tensor(cejunk, ysb, pl4, ALU.mult)
            nc.vector.tensor_reduce(ceacc, cejunk, AX.X, ALU.add)
            lns4 = sp.tile([128, 1], f32, name="lns4", tag="lns4")
            nc.scalar.activation(lns4, s4, AF.Ln)
            ceb = sp.tile([128, 1], f32, name="ceb", tag="ceb")
            nc.vector.tensor_tensor(ceb, ceacc, lns4, ALU.subtract)

            nst = nwin * WWIN
            lnse = sp.tile([128, nst], f32, name="lnse", tag="lnse")
            nc.scalar.activation(lnse, sestash[:, 0:nst], AF.Ln)
            lnsp = sp.tile([128, nst], f32, name="lnsp", tag="lnsp")
            nc.scalar.activation(lnsp, spstash[:, 0:nst], AF.Ln)
            d3 = sp.tile([128, nst], f32, name="d3", tag="d3")
            nc.vector.tensor_tensor(d3, lnsp, lnse, ALU.subtract)
            r3 = sp.tile([128, 1], f32, name="r3", tag="r3")
            nc.vector.tensor_reduce(r3, d3, AX.X, ALU.add)
            klb = sp.tile([128, 1], f32, name="klb", tag="klb")
            nc.vector.tensor_tensor(klb, klacc, r3, ALU.add)

            nc.sync.dma_start(out=out_d[0, 0:64], in_=acc1)
            nc.sync.dma_start(out=out_d[0, 64:128], in_=acc2)
            nc.sync.dma_start(out=out_d[1, :], in_=klb)
            nc.sync.dma_start(out=out_d[2, :], in_=ceb)

    nc.finalize()
    return nc


# =====================================================================
# host side
# =====================================================================

def _relu_np(a):
    return np.maximum(a, 0.0)


def _pack_consts(d):
    """name->array dict  ->  (wbf [128,CB] bf16, wf [128,CF] f32)."""
    lay, ncol_bf, ncol_f32 = _pack_layout()
    wbf = np.zeros((128, ncol_bf), BF16)
    wf = np.zeros((128, ncol_f32), F32)
    for name, arr in d.items():
        p, c, tag, off = lay[name]
        assert arr.shape == (p, c), (name, arr.shape, (p, c))
        if tag == "bf":
            wbf[0:p, off:off + c] = arr
        else:
            wf[0:p, off:off + c] = arr
    return wbf, wf


def _prep_weights(inputs):
    """Build the per-core (shared) weight arrays dict name->np array."""
    c_means = _f32(inputs["c_means"])
    p_enc = [_f32(a) for a in inputs["p_enc"]]
    p_prior = [_f32(a) for a in inputs["p_prior"]]
    p_phix = [_f32(a) for a in inputs["p_phix"]]
    p_phiz = [_f32(a) for a in inputs["p_phiz"]]
    p_cell = [_f32(a) for a in inputs["p_cell"]]
    p_pred = [_f32(a) for a in inputs["p_pred"]]
    dec = [_f32(a) for a in inputs["dec"]]
    Wih, Whh, bl, Wout, bout = dec

    d = {}

    def mlp_common(tag, W2, b2):
        d[f"w2{tag}_0"] = _bf(W2[0:128])
        d[f"w2{tag}_1"] = _bf(W2[128:256])
        d[f"b2{tag}"] = _f32(np.stack([b2[0:128], b2[128:256]], axis=1))

    # phix
    W1, b1, W2, b2, W3, b3 = p_phix
    d["w1x_aug"] = _bf(np.vstack([W1, b1[None, :]]))
    mlp_common("x", W2, b2)
    z65 = np.zeros((128, 1), F32)
    d["w3x_0"] = _bf(np.hstack([W3[0:128], z65]))
    d["w3x_1"] = _bf(np.hstack([W3[128:256], z65]))
    d["b3x_aug"] = _f32(np.concatenate([b3, [1.0]])[:, None])
    # prior
    W1, b1, W2, b2, W3, b3 = p_prior
    d["w1p_aug"] = _bf(np.vstack([W1, b1[None, :]]))
    mlp_common("p", W2, b2)
    d["w3p_0"] = _bf(W3[0:128])
    d["w3p_1"] = _bf(W3[128:256])
    d["b3p_r"] = _bf(b3[None, :])
    # enc
    W1, b1, W2, b2, W3, b3 = p_enc
    d["w1e_x"] = _bf(np.vstack([W1[0:64], b1[None, :]]))
    d["w1e_h"] = _bf(W1[64:128])
    mlp_common("e", W2, b2)
    d["w3e_0"] = _bf(W3[0:128])
    d["w3e_1"] = _bf(W3[128:256])
    d["b3e_r"] = _bf(b3[None, :])
    # phiz
    W1, b1, W2, b2, W3, b3 = p_phiz
    d["w1z_aug"] = _bf(np.vstack([W1, b1[None, :]]))
    mlp_common("z", W2, b2)
    d["w3z_0"] = _bf(np.hstack([W3[0:128], z65]))
    d["w3z_1"] = _bf(np.hstack([W3[128:256], z65]))
    d["b3z_aug"] = _f32(np.concatenate([b3, [1.0]])[:, None])
    # cell (input order: h | phx | u)
    W1, b1, W2, b2, W3, b3 = p_cell
    d["w1c_h"] = _bf(W1[0:64])
    d["w1c_x"] = _bf(np.vstack([W1[64:128], b1[None, :]]))
    d["w1c_u"] = _bf(W1[128:192])
    mlp_common("c", W2, b2)
    d["w3c_0"] = _bf(np.hstack([W3[0:128], z65]))
    d["w3c_1"] = _bf(np.hstack([W3[128:256], z65]))
    d["b3c_aug"] = _f32(np.concatenate([b3, [1.0]])[:, None])
    # pred
    W1, b1, W2, b2, W3, b3 = p_pred
    d["w1d_aug"] = _bf(np.vstack([W1, b1[None, :]]))
    mlp_common("d", W2, b2)
    d["w3d_0"] = _bf(W3[0:128])
    d["w3d_1"] = _bf(W3[128:256])
    d["b3d_r"] = _bf(b3[None, :])
    # decoder; reorder gate blocks [i f g o] -> [i f o g]
    perm = np.r_[0:128, 128:256, 384:512, 256:384]
    wih_r = Wih[:, perm].copy()
    whh_r = Whh[:, perm].copy()
    bl_r = bl[perm].copy()
    # g-gate block pre-doubled so one tanh(psum/2) covers all four gates
    wih_r[:, 384:512] *= 2.0
    whh_r[:, 384:512] *= 2.0
    bl_r[384:512] *= 2.0
    d["wih_u"] = _bf(np.vstack([wih_r[0:64], bl_r[None, :]]))
    d["wih_h"] = _bf(wih_r[64:128])
    d["whh0_u"] = _bf(whh_r[0:64])
    d["whh0_h"] = _bf(whh_r[64:128])
    d["whh_full"] = _bf(whh_r)
    d["wout_mu"] = _bf(Wout[:, 0:64])
    d["wout_lv"] = _bf(Wout[:, 64:128])
    d["nlv2"] = _f32(-0.5 * bout[64:128][:, None])
    d["bmu"] = _f32(bout[0:64][:, None])
    # misc
    d["cma"] = _bf(np.hstack([c_means, np.ones((KCL, 1), F32)]))
    d["i128"] = _bf(np.eye(128, dtype=F32))
    d["identf"] = _f32(np.eye(128, dtype=F32))
    d["ones1"] = _bf(np.ones((1, 128), F32))
    # initial carries
    z0 = c_means.mean(axis=0)
    W1, b1, W2, b2, W3, b3 = p_phiz
    u0 = _relu_np(_relu_np(_relu_np(z0 @ W1 + b1) @ W2 + b2) @ W3 + b3)
    u0a = np.concatenate([u0, [1.0]]).astype(F32)
    d["u0b"] = _bf(np.broadcast_to(u0a[:, None], (65, BL)))
    h0a = np.zeros(65, F32)
    h0a[64] = 1.0
    d["h0b"] = _bf(np.broadcast_to(h0a[:, None], (65, BL)))
    return d, bout


def _prep_x(x, nwin):
    """x (B, S, D) f32 -> per-core list of (xf [nwin,64,NT], xbf [nwin,65,NT])."""
    res = []
    for c in range(NCORES):
        xc = np.asarray(x[c * BL:(c + 1) * BL], dtype=F32)  # (BL, S, D)
        xw = xc.reshape(BL, NWIN_FULL, WWIN, D)[:, :nwin]
        # -> (nwin, D, WWIN, BL) -> (nwin, D, NT)
        xt = np.ascontiguousarray(xw.transpose(1, 3, 2, 0).reshape(nwin, D, WWIN * BL))
        ones = np.ones((nwin, 1, WWIN * BL), F32)
        xbf = np.concatenate([xt, ones], axis=1).astype(BF16)
        res.append((xt, np.ascontiguousarray(xbf)))
    return res


def kernel(**inputs):
    global LAST_RESULTS
    from concourse.bass_utils import run_bass_kernel_spmd

    nwin = int(os.environ.get("DIRVRNN_NWIN", NWIN_FULL))
    trace = os.environ.get("DIRVRNN_TRACE", "0") == "1"

    if nwin not in _CACHE:
        _CACHE[nwin] = _build(nwin)
    nc = _CACHE[nwin]

    wdict, bout = _prep_weights(inputs)
    xs = _prep_x(np.asarray(inputs["x"], dtype=F32), nwin)
    y = _f32(inputs["y"])

    in_maps = []
    for c in range(NCORES):
        d = dict(wdict)
        d["yin"] = np.ascontiguousarray(y[c * BL:(c + 1) * BL])
        wbf, wf = _pack_consts(d)
        m = {"wbf": wbf, "wf": wf}
        m["xf"], m["xbf"] = xs[c]
        in_maps.append(m)

    res = run_bass_kernel_spmd(nc, in_maps, core_ids=list(range(NCORES)),
                               trace=trace)
    LAST_RESULTS = res

    S1 = S2 = Skl = Sce = 0.0
    for r in res.results:
        o = np.asarray(r["out"], dtype=np.float64)
        S1 += o[0, 0:64].sum()
        S2 += o[0, 64:128].sum()
        Skl += o[1].sum()
        Sce += o[2].sum()

    b_lv = np.asarray(bout[64:128], dtype=np.float64)
    nsteps = nwin * WWIN
    loss = (0.5 * (S1 + S2) / B
            + 0.5 * nsteps * b_lv.sum()
            + 0.5 * LOG2PI * D * nsteps
            + Skl / B
            - Sce / B)
    return np.float32(loss)


# revision 29
# speedup vs baseline: 1.0511x; 1.0511x over previous
"""DirVRNN loss kernel for 8 Trainium2 NeuronCores (Bass/Tile).

Strategy: pure data parallel. Batch 1024 -> 8 cores x 128. Per core the whole
recurrence (32 windows x 16 steps + per-window 16-step LSTM decoder) runs
on-chip with feature-major activations [feat, batch=128], bf16 matmuls with
fp32 PSUM accumulation. Each core emits partial sums; the host combines them
into the scalar loss.
"""

import os
import sys

import numpy as np

for _p in ("/opt/trn_rl_repo",):
    if _p not in sys.path and os.path.isdir(_p):
        sys.path.insert(0, _p)

import ml_dtypes

BF16 = ml_dtypes.bfloat16

# ---------------- problem dims (hardcoded per spec) ----------------
B, S, D = 1024, 512, 64
L, KCL, NCLS = 64, 16, 4
NH = 256
WWIN = 16
H2 = 2 * L            # 128
NCORES = 8
BL = B // NCORES      # 128 batch per core
NWIN_FULL = S // WWIN  # 32
NT = WWIN * BL        # 2048 columns per window
LOG2PI = float(np.log(2.0 * np.pi))

F32 = np.float32

_CACHE = {}
LAST_RESULTS = None  # BassKernelResults of last run (for test.py)


def _bf(a):
    return np.ascontiguousarray(np.asarray(a, dtype=F32).astype(BF16))


def _f32(a):
    return np.ascontiguousarray(np.asarray(a, dtype=F32))


# ---------------- packed-constant layout (shared host/device) ----------------
# name -> (partitions, cols, dtype_tag)  dtype_tag in {"bf", "f32"}
def _const_specs():
    sp = {}
    for mlp in ("x", "p", "e", "z", "c", "d"):
        sp[f"w2{mlp}_0"] = (128, 256, "bf")
        sp[f"w2{mlp}_1"] = (128, 256, "bf")
        sp[f"b2{mlp}"] = (128, 2, "f32")
    sp.update({
        "w1x_aug": (65, 256, "bf"),
        "w3x_0": (128, 65, "bf"), "w3x_1": (128, 65, "bf"),
        "b3x_aug": (65, 1, "f32"),
        "w1p_aug": (65, 256, "bf"),
        "w3p_0": (128, 16, "bf"), "w3p_1": (128, 16, "bf"),
        "b3p_r": (1, 16, "bf"),
        "w1e_x": (65, 256, "bf"), "w1e_h": (64, 256, "bf"),
        "w3e_0": (128, 16, "bf"), "w3e_1": (128, 16, "bf"),
        "b3e_r": (1, 16, "bf"),
        "w1z_aug": (65, 256, "bf"),
        "w3z_0": (128, 65, "bf"), "w3z_1": (128, 65, "bf"),
        "b3z_aug": (65, 1, "f32"),
        "w1c_h": (64, 256, "bf"), "w1c_x": (65, 256, "bf"),
        "w1c_u": (64, 256, "bf"),
        "w3c_0": (128, 65, "bf"), "w3c_1": (128, 65, "bf"),
        "b3c_aug": (65, 1, "f32"),
        "w1d_aug": (65, 256, "bf"),
        "w3d_0": (128, 4, "bf"), "w3d_1": (128, 4, "bf"),
        "b3d_r": (1, 4, "bf"),
        "wih_u": (65, 512, "bf"), "wih_h": (64, 512, "bf"),
        "whh0_u": (64, 512, "bf"), "whh0_h": (64, 512, "bf"),
        "whh_full": (128, 512, "bf"),
        "wout_mu": (128, 64, "bf"), "wout_lv": (128, 64, "bf"),
        "nlv2": (64, 1, "f32"), "bmu": (64, 1, "f32"),
        "cma": (16, 65, "bf"),
        "i128": (128, 128, "bf"),
        "identf": (128, 128, "f32"),
        "ones1": (1, 128, "bf"),
        "u0b": (65, BL, "bf"), "h0b": (65, BL, "bf"),
        "yin": (BL, NCLS, "f32"),
    })
    return sp


def _pack_layout():
    """Deterministic column layout of the two constant packs."""
    specs = _const_specs()
    lay = {}
    off = {"bf": 0, "f32": 0}
    for name in sorted(specs):
        p, c, tag = specs[name]
        lay[name] = (p, c, tag, off[tag])
        off[tag] += (c + 7) // 8 * 8
    return lay, off["bf"], off["f32"]


# =====================================================================
# device program
# =====================================================================

def _build(nwin, parts=("phix", "dec", "inner", "ep")):
    import concourse.bass as bass
    import concourse.tile as tile
    from concourse import bacc, mybir

    AF = mybir.ActivationFunctionType
    ALU = mybir.AluOpType
    AX = mybir.AxisListType
    f32 = mybir.dt.float32
    bf16 = mybir.dt.bfloat16

    nc = bacc.Bacc("TRN2", target_bir_lowering=False, detect_race_conditions=False)

    # ---------------- dram inputs ----------------
    lay, ncol_bf, ncol_f32 = _pack_layout()
    wbf_d = nc.dram_tensor("wbf", [128, ncol_bf], bf16, kind="ExternalInput")
    wf_d = nc.dram_tensor("wf", [128, ncol_f32], f32, kind="ExternalInput")
    xf_d = nc.dram_tensor("xf", [nwin, 64, NT], f32, kind="ExternalInput")
    xb_d = nc.dram_tensor("xbf", [nwin, 65, NT], bf16, kind="ExternalInput")
    out_d = nc.dram_tensor("out", [3, 128], f32, kind="ExternalOutput")

    with tile.TileContext(nc) as tc:
        from contextlib import ExitStack

        with ExitStack() as ctx:
            wp = ctx.enter_context(tc.tile_pool(name="wp", bufs=1))
            xp = ctx.enter_context(tc.tile_pool(name="xp", bufs=3))
            ap = ctx.enter_context(tc.tile_pool(name="ap", bufs=6))   # per-step acts
            cp = ctx.enter_context(tc.tile_pool(name="cp", bufs=6))   # carries
            sp = ctx.enter_context(tc.tile_pool(name="sp", bufs=1))   # persistent stash
            psA = ctx.enter_context(tc.tile_pool(name="psA", bufs=3, space="PSUM"))
            psB = ctx.enter_context(tc.tile_pool(name="psB", bufs=2, space="PSUM"))
            psC = ctx.enter_context(tc.tile_pool(name="psC", bufs=1, space="PSUM"))
            psD = ctx.enter_context(tc.tile_pool(name="psD", bufs=1, space="PSUM"))
            psE = ctx.enter_context(tc.tile_pool(name="psE", bufs=1, space="PSUM"))

            wbf_t = wp.tile([128, ncol_bf], bf16, name="wbf_t", tag="wbf_t")
            nc.sync.dma_start(out=wbf_t, in_=wbf_d[:, :])
            wf_t = wp.tile([128, ncol_f32], f32, name="wf_t", tag="wf_t")
            nc.sync.dma_start(out=wf_t, in_=wf_d[:, :])
            W = {}
            for name, (p, c, tag, off) in lay.items():
                src = wbf_t if tag == "bf" else wf_t
                W[name] = src[0:p, off:off + c]
            ysb = W["yin"]

            # warm-up: make every engine observe each pack's DMA semaphore
            # before any real op, so no compute op carries >1 DMA wait.
            wu = wp.tile([1, 16], f32, name="wu", tag="wu")
            nc.vector.tensor_copy(wu, wf_t[0:1, 0:16])
            nc.vector.tensor_copy(wu, wbf_t[0:1, 0:16])
            nc.scalar.activation(wu, wf_t[0:1, 0:16], AF.Copy)
            nc.scalar.activation(wu, wbf_t[0:1, 0:16], AF.Copy)

            # persistent accumulators / stashes
            sestash = sp.tile([128, S], f32, name="sestash", tag="sestash")
            spstash = sp.tile([128, S], f32, name="spstash", tag="spstash")
            klacc = sp.tile([128, 1], f32, name="klacc", tag="klacc")
            acc1 = sp.tile([64, 1], f32, name="acc1", tag="acc1")
            acc2 = sp.tile([64, 1], f32, name="acc2", tag="acc2")
            nc.vector.memset(klacc, 0.0)
            nc.vector.memset(acc1, 0.0)
            nc.vector.memset(acc2, 0.0)

            # initial carries (read-only slices of the constant pack)
            u_ap = W["u0b"]
            h_ap = W["h0b"]

            mmop = nc.tensor.matmul

            def relu_evac(eng, out, ps, bias=None):
                if eng == "s":
                    nc.scalar.activation(out, ps, AF.Relu,
                                         bias=bias if bias is not None else 0.0)
                else:
                    if bias is None:
                        nc.vector.tensor_scalar(out, ps, 0.0, None, ALU.max)
                    else:
                        nc.vector.tensor_scalar(out, ps, bias, 0.0, ALU.add, ALU.max)

            def copy_evac(eng, out, ps):
                if eng == "s":
                    nc.scalar.activation(out, ps, AF.Copy)
                else:
                    nc.vector.tensor_copy(out, ps)

            def relu_evac2(out, ps, ncols, bias0=None, bias1=None):
                """Split a [P, ncols] relu-evac across ACT and DVE."""
                h = ncols // 2
                if bias0 is None:
                    nc.scalar.activation(out[:, 0:h], ps[:, 0:h], AF.Relu)
                    nc.vector.tensor_scalar(out[:, h:ncols], ps[:, h:ncols],
                                            0.0, None, ALU.max)
                else:
                    nc.scalar.activation(out[:, 0:h], ps[:, 0:h], AF.Relu,
                                         bias=bias0)
                    nc.vector.tensor_scalar(out[:, h:ncols], ps[:, h:ncols],
                                            bias1, 0.0, ALU.add, ALU.max)

            # engine assignment per op site ("s"=ScalarE/ACT, "v"=VectorE/DVE)
            E = {
                "enc1": "s", "enc2": "v", "prior1": "s", "prior2": "v",
                "phiz1": "v", "phiz2": "v", "phiz3": "s",
                "cell1": "s", "cell2": "s", "cell3": "s",
                "px1": "s", "px2": "v", "px3": "s",
                "piT": "v", "zev": "s", "cwev": "s", "lpev": "s",
            }

            ts = bass.ts

            def dma_window(w):
                xf_t = xp.tile([64, NT], f32, name="xf_t", tag="xf_t")
                nc.sync.dma_start(out=xf_t, in_=xf_d[w, :, :])
                xbf_t = xp.tile([65, NT], bf16, name="xbf_t", tag="xbf_t")
                nc.sync.dma_start(out=xbf_t, in_=xb_d[w, :, :])
                xb_t = xp.tile([64, NT], f32, name="xb_t", tag="xb_t")
                nc.vector.tensor_scalar(xb_t, xf_t, W["bmu"][:, 0:1], None,
                                        ALU.subtract)
                phx_t = xp.tile([65, NT], bf16, name="phx_t", tag="phx_t")
                return xbf_t, xb_t, phx_t

            def phix_chunk(nch, xbf_t, phx_t):
                cs = slice(nch * 512, (nch + 1) * 512)
                p1a = psB.tile([128, 512], f32, name="p1a", tag="psB")
                mmop(p1a, W["w1x_aug"][:, 0:128], xbf_t[:, cs],
                     start=True, stop=True)
                p1b = psB.tile([128, 512], f32, name="p1b", tag="psB")
                mmop(p1b, W["w1x_aug"][:, 128:256], xbf_t[:, cs],
                     start=True, stop=True)
                ph1 = xp.tile([128, 1024], bf16, name="ph1", tag="ph1")
                relu_evac(E["px1"], ph1[:, 0:512], p1a)
                relu_evac(E["px1"], ph1[:, 512:1024], p1b)
                p2a = psB.tile([128, 512], f32, name="p2a", tag="psB")
                mmop(p2a, W["w2x_0"][:, 0:128], ph1[:, 0:512],
                     start=True, stop=False)
                mmop(p2a, W["w2x_1"][:, 0:128], ph1[:, 512:1024],
                     start=False, stop=True)
                p2b = psB.tile([128, 512], f32, name="p2b", tag="psB")
                mmop(p2b, W["w2x_0"][:, 128:256], ph1[:, 0:512],
                     start=True, stop=False)
                mmop(p2b, W["w2x_1"][:, 128:256], ph1[:, 512:1024],
                     start=False, stop=True)
                ph2 = xp.tile([128, 1024], bf16, name="ph2", tag="ph2")
                if E["px2"] == "s":
                    nc.scalar.activation(ph2[:, 0:512], p2a, AF.Relu,
                                         bias=W["b2x"][:, 0:1])
                    nc.scalar.activation(ph2[:, 512:1024], p2b, AF.Relu,
                                         bias=W["b2x"][:, 1:2])
                else:
                    nc.vector.tensor_scalar(ph2[:, 0:512], p2a,
                                            W["b2x"][:, 0:1], 0.0,
                                            ALU.add, ALU.max)
                    nc.vector.tensor_scalar(ph2[:, 512:1024], p2b,
                                            W["b2x"][:, 1:2], 0.0,
                                            ALU.add, ALU.max)
                p3 = psB.tile([65, 512], f32, name="p3", tag="psB")
                mmop(p3, W["w3x_0"], ph2[:, 0:512], start=True, stop=False)
                mmop(p3, W["w3x_1"], ph2[:, 512:1024], start=False, stop=True)
                relu_evac(E["px3"], phx_t[:, cs], p3, bias=W["b3x_aug"][:, 0:1])

            # prologue: window 0 x + phix
            xbf_cur, xb_cur, phx_cur = dma_window(0)
            for nch in range(4):
                phix_chunk(nch, xbf_cur, phx_cur)

            for w in range(nwin):
                if w + 1 < nwin:
                    xbf_nxt, xb_nxt, phx_nxt = dma_window(w + 1)
                phx = phx_cur
                xb_t = xb_cur
                u_prev, h_prev = u_ap, h_ap

                # ---- decoder: cw = ctx @ Wih + bl (hoisted once per window) ----
                pcw = psA.tile([128, 512], f32, name="pcw", tag="psA")
                for g in range(4):
                    gs = slice(g * 128, (g + 1) * 128)
                    mmop(pcw[:, gs], W["wih_u"][:, gs], u_prev[0:65, :],
                         start=True, stop=False)
                    mmop(pcw[:, gs], W["wih_h"][:, gs], h_prev[0:64, :],
                         start=False, stop=True)
                cw = cp.tile([128, 512], bf16, name="cw", tag="cw")
                copy_evac(E["cwev"], cw, pcw)

                # ---- logits psum stash for the window ----
                pl = psD.tile([128, 512], f32, name="pl", tag="psD")
                le = pl[:, 0:256]
                lp = pl[:, 256:512]
                pist = xp.tile([128, 256], f32, name="pist", tag="pist")

                hd_prev = None
                cd_prev = None
                hd4 = None

                # ---- inner recurrence, decoder + next-window phix interleaved
                for t in range(WWIN):
                    tgcol = w * WWIN + t
                    # enc L1
                    pe1 = psA.tile([128, 256], f32, name="pe1", tag="psA")
                    for m in range(2):
                        ms = slice(m * 128, (m + 1) * 128)
                        mmop(pe1[:, ms], W["w1e_x"][:, ms], phx[:, ts(t, 128)],
                             start=True, stop=False)
                        mmop(pe1[:, ms], W["w1e_h"][:, ms], h_ap[0:64, :],
                             start=False, stop=True)
                    h1e = ap.tile([128, 256], bf16, name="h1e", tag="h1e")
                    relu_evac(E["enc1"], h1e, pe1)
                    # prior L1
                    pp1 = psA.tile([128, 256], f32, name="pp1", tag="psA")
                    for m in range(2):
                        ms = slice(m * 128, (m + 1) * 128)
                        mmop(pp1[:, ms], W["w1p_aug"][:, ms], h_ap[0:65, :],
                             start=True, stop=True)
                    h1p = ap.tile([128, 256], bf16, name="h1p", tag="h1p")
                    relu_evac(E["prior1"], h1p, pp1)
                    # enc L2
                    pe2 = psA.tile([128, 256], f32, name="pe2", tag="psA")
                    for m in range(2):
                        ms = slice(m * 128, (m + 1) * 128)
                        mmop(pe2[:, ms], W["w2e_0"][:, ms], h1e[:, 0:128],
                             start=True, stop=False)
                        mmop(pe2[:, ms], W["w2e_1"][:, ms], h1e[:, 128:256],
                             start=False, stop=True)
                    h2e = ap.tile([128, 256], bf16, name="h2e", tag="h2e")
                    for m in range(2):
                        ms = slice(m * 128, (m + 1) * 128)
                        relu_evac(E["enc2"], h2e[:, ms], pe2[:, ms],
                                  bias=W["b2e"][:, m:m + 1])
                    # enc logits (batch-major)
                    lesl = le[:, t * 16:(t + 1) * 16]
                    mmop(lesl, W["ones1"], W["b3e_r"], start=True, stop=False)
                    mmop(lesl, h2e[:, 0:128], W["w3e_0"], start=False,
                         stop=False)
                    mmop(lesl, h2e[:, 128:256], W["w3e_1"], start=False,
                         stop=True)
                    # softmax -> pi, z
                    ep = ap.tile([128, 16], f32, name="ep", tag="ep")
                    nc.scalar.activation(ep, lesl, AF.Exp)
                    nc.vector.reduce_sum(sestash[:, tgcol:tgcol + 1], ep, AX.X)
                    rcp = ap.tile([128, 1], f32, name="rcp", tag="rcp")
                    nc.vector.reciprocal(rcp, sestash[:, tgcol:tgcol + 1])
                    pisl = pist[:, t * 16:(t + 1) * 16]
                    nc.vector.tensor_scalar(pisl, ep, rcp, None, ALU.mult)
                    pz = psE.tile([128, 256], f32, name="pz", tag="psE")
                    nc.tensor.transpose(pz[0:16, 0:128], pisl, W["identf"])
                    piT = ap.tile([16, 128], bf16, name="piT", tag="piT")
                    copy_evac(E["piT"], piT, pz[0:16, 0:128])
                    mmop(pz[0:65, 128:256], W["cma"], piT, start=True,
                         stop=True)
                    z_ap = cp.tile([65, BL], bf16, name="z_ap", tag="z")
                    copy_evac(E["zev"], z_ap, pz[0:65, 128:256])
                    # phiz
                    pz1 = psA.tile([128, 256], f32, name="pz1", tag="psA")
                    for m in range(2):
                        ms = slice(m * 128, (m + 1) * 128)
                        mmop(pz1[:, ms], W["w1z_aug"][:, ms], z_ap[0:65, :],
                             start=True, stop=True)
                    h1z = ap.tile([128, 256], bf16, name="h1z", tag="h1z")
                    relu_evac(E["phiz1"], h1z, pz1)
                    pz2 = psA.tile([128, 256], f32, name="pz2", tag="psA")
                    for m in range(2):
                        ms = slice(m * 128, (m + 1) * 128)
                        mmop(pz2[:, ms], W["w2z_0"][:, ms], h1z[:, 0:128],
                             start=True, stop=False)
                        mmop(pz2[:, ms], W["w2z_1"][:, ms], h1z[:, 128:256],
                             start=False, stop=True)
                    h2z = ap.tile([128, 256], bf16, name="h2z", tag="h2z")
                    for m in range(2):
                        ms = slice(m * 128, (m + 1) * 128)
                        relu_evac(E["phiz2"], h2z[:, ms], pz2[:, ms],
                                  bias=W["b2z"][:, m:m + 1])
                    pz3 = psA.tile([65, 128], f32, name="pz3", tag="psA")
                    mmop(pz3, W["w3z_0"], h2z[:, 0:128], start=True, stop=False)
                    mmop(pz3, W["w3z_1"], h2z[:, 128:256], start=False,
                         stop=True)
                    u_new = cp.tile([65, BL], bf16, name="u_new", tag="u")
                    relu_evac(E["phiz3"], u_new, pz3, bias=W["b3z_aug"][:, 0:1])
                    # prior L2
                    pp2 = psA.tile([128, 256], f32, name="pp2", tag="psA")
                    for m in range(2):
                        ms = slice(m * 128, (m + 1) * 128)
                        mmop(pp2[:, ms], W["w2p_0"][:, ms], h1p[:, 0:128],
                             start=True, stop=False)
                        mmop(pp2[:, ms], W["w2p_1"][:, ms], h1p[:, 128:256],
                             start=False, stop=True)
                    h2p = ap.tile([128, 256], bf16, name="h2p", tag="h2p")
                    for m in range(2):
                        ms = slice(m * 128, (m + 1) * 128)
                        relu_evac(E["prior2"], h2p[:, ms], pp2[:, ms],
                                  bias=W["b2p"][:, m:m + 1])
                    lpsl = lp[:, t * 16:(t + 1) * 16]
                    mmop(lpsl, W["ones1"], W["b3p_r"], start=True, stop=False)
                    mmop(lpsl, h2p[:, 0:128], W["w3p_0"], start=False,
                         stop=False)
                    mmop(lpsl, h2p[:, 128:256], W["w3p_1"], start=False,
                         stop=True)
                    # cell
                    pc1 = psA.tile([128, 256], f32, name="pc1", tag="psA")
                    for m in range(2):
                        ms = slice(m * 128, (m + 1) * 128)
                        mmop(pc1[:, ms], W["w1c_x"][:, ms], phx[:, ts(t, 128)],
                             start=True, stop=False)
                        mmop(pc1[:, ms], W["w1c_h"][:, ms], h_ap[0:64, :],
                             start=False, stop=False)
                        mmop(pc1[:, ms], W["w1c_u"][:, ms], u_new[0:64, :],
                             start=False, stop=True)
                    h1c = ap.tile([128, 256], bf16, name="h1c", tag="h1c")
                    relu_evac(E["cell1"], h1c, pc1)
                    pc2 = psA.tile([128, 256], f32, name="pc2", tag="psA")
                    for m in range(2):
                        ms = slice(m * 128, (m + 1) * 128)
                        mmop(pc2[:, ms], W["w2c_0"][:, ms], h1c[:, 0:128],
                             start=True, stop=False)
                        mmop(pc2[:, ms], W["w2c_1"][:, ms], h1c[:, 128:256],
                             start=False, stop=True)
                    h2c = ap.tile([128, 256], bf16, name="h2c", tag="h2c")
                    for m in range(2):
                        ms = slice(m * 128, (m + 1) * 128)
                        relu_evac(E["cell2"], h2c[:, ms], pc2[:, ms],
                                  bias=W["b2c"][:, m:m + 1])
                    pc3 = psA.tile([65, 128], f32, name="pc3", tag="psA")
                    mmop(pc3, W["w3c_0"], h2c[:, 0:128], start=True, stop=False)
                    mmop(pc3, W["w3c_1"], h2c[:, 128:256], start=False,
                         stop=True)
                    h_new = cp.tile([65, BL], bf16, name="h_new", tag="h")
                    relu_evac(E["cell3"], h_new, pc3, bias=W["b3c_aug"][:, 0:1])

                    u_ap = u_new
                    h_ap = h_new

                    # ---- decoder LSTM step j = t (independent chain) ----
                    j = t
                    if j % 4 == 0:
                        hd4 = cp.tile([128, 512], bf16, name="hd4", tag="hd4",
                                      bufs=2)
                    pg = psC.tile([128, 512], f32, name="pg", tag="psC")
                    for g in range(4):
                        gs = slice(g * 128, (g + 1) * 128)
                        if j == 0:
                            mmop(pg[:, gs], W["whh0_u"][:, gs], u_prev[0:64, :],
                                 start=True, stop=False)
                            mmop(pg[:, gs], W["whh0_h"][:, gs], h_prev[0:64, :],
                                 start=False, stop=False)
                        else:
                            mmop(pg[:, gs], W["whh_full"][:, gs], hd_prev,
                                 start=True, stop=False)
                    mmop(pg, W["i128"], cw, start=False, stop=True,
                         skip_group_check=True)
                    # g-gate weights are pre-doubled: one tanh(psum/2) does
                    # both the 3 sigmoids' halves and the raw g tanh.
                    tall = ap.tile([128, 512], bf16, name="tall", tag="tall",
                                   bufs=2)
                    nc.scalar.activation(tall, pg, AF.Tanh, scale=0.5)
                    tg = tall[:, 384:512]
                    # sigmoid(g) = 0.5*tanh(g/2) + 0.5
                    sig = ap.tile([128, 384], bf16, name="sig", tag="sig",
                                  bufs=2)
                    nc.vector.tensor_scalar(sig, tall[:, 0:384], 0.5, 0.5,
                                            ALU.mult, ALU.add)
                    t1 = ap.tile([128, 128], bf16, name="t1", tag="t1", bufs=2)
                    nc.vector.tensor_tensor(t1, sig[:, 0:128], tg, ALU.mult)
                    if j == 0:
                        cd = t1
                    else:
                        t2 = ap.tile([128, 128], bf16, name="t2", tag="t2",
                                     bufs=2)
                        nc.vector.tensor_tensor(t2, sig[:, 128:256], cd_prev,
                                                ALU.mult)
                        cd = ap.tile([128, 128], bf16, name="cd", tag="cd",
                                     bufs=2)
                        nc.vector.tensor_tensor(cd, t1, t2, ALU.add)
                    cd_prev = cd
                    tcd = ap.tile([128, 128], bf16, name="tcd", tag="tcd",
                                  bufs=2)
                    nc.scalar.activation(tcd, cd, AF.Tanh)
                    hs = slice((j % 4) * 128, (j % 4 + 1) * 128)
                    nc.vector.tensor_tensor(hd4[:, hs], sig[:, 256:384], tcd,
                                            ALU.mult)
                    hd_prev = hd4[:, hs]

                    if j % 4 == 3:
                        g4 = j // 4
                        pmu = psB.tile([64, 512], f32, name="pmu", tag="psB")
                        mmop(pmu, W["wout_mu"], hd4, start=True, stop=True)
                        plv = psB.tile([64, 512], f32, name="plv", tag="psB")
                        mmop(plv, W["wout_lv"], hd4, start=True, stop=True)
                        e2 = ap.tile([64, 512], f32, name="e2", tag="e2", bufs=2)
                        nc.scalar.activation(e2, plv, AF.Exp, scale=-0.5,
                                             bias=W["nlv2"][:, 0:1])
                        dt_ = ap.tile([64, 512], f32, name="dt_", tag="dt_",
                                      bufs=2)
                        nc.vector.tensor_tensor(
                            dt_, xb_t[:, g4 * 512:(g4 + 1) * 512], pmu,
                            ALU.subtract)
                        qt = ap.tile([64, 512], f32, name="qt", tag="qt", bufs=2)
                        nc.vector.tensor_tensor(qt, dt_, e2, ALU.mult)
                        jq = ap.tile([64, 512], f32, name="jq", tag="jq", bufs=2)
                        sq = ap.tile([64, 1], f32, name="sq", tag="sq", bufs=2)
                        nc.scalar.activation(jq, qt, AF.Square, accum_out=sq)
                        nc.vector.tensor_tensor(acc1, acc1, sq, ALU.add)
                        lvs = ap.tile([64, 1], f32, name="lvs", tag="lvs",
                                      bufs=2)
                        nc.vector.tensor_reduce(lvs, plv, AX.X, ALU.add)
                        nc.vector.tensor_tensor(acc2, acc2, lvs, ALU.add)

                    # ---- spread next window's phix across this window ----
                    if w + 1 < nwin and t in (2, 6, 10, 14):
                        phix_chunk((t - 2) // 4, xbf_nxt, phx_nxt)

                # ---- KL for the window ----
                lpsb = ap.tile([128, 256], f32, name="lpsb", tag="lpsb", bufs=2)
                copy_evac(E["lpev"], lpsb, lp)
                epp = ap.tile([128, 256], f32, name="epp", tag="epp", bufs=2)
                nc.scalar.activation(epp, lp, AF.Exp)
                nc.vector.tensor_reduce(
                    spstash[:, w * 16:(w + 1) * 16],
                    epp.rearrange("p (t k) -> p t k", k=16), AX.X, ALU.add)
                dif = ap.tile([128, 256], f32, name="dif", tag="dif", bufs=2)
                nc.vector.tensor_tensor(dif, le, lpsb, ALU.subtract)
                jkl = ap.tile([128, 256], f32, name="jkl", tag="jkl", bufs=2)
                nc.vector.tensor_tensor(jkl, pist, dif, ALU.mult)
                rkl = ap.tile([128, 1], f32, name="rkl", tag="rkl", bufs=2)
                nc.vector.tensor_reduce(rkl, jkl, AX.X, ALU.add)
                nc.vector.tensor_tensor(klacc, klacc, rkl, ALU.add)

                if w + 1 < nwin:
                    xbf_cur, xb_cur, phx_cur = xbf_nxt, xb_nxt, phx_nxt

            # ================= epilogue =================
            pd1 = psA.tile([128, 256], f32, name="pd1", tag="psA")
            for m in range(2):
                ms = slice(m * 128, (m + 1) * 128)
                mmop(pd1[:, ms], W["w1d_aug"][:, ms], z_ap[0:65, :],
                     start=True, stop=True)
            h1d = ap.tile([128, 256], bf16, name="h1d", tag="h1d")
            relu_evac("v", h1d, pd1)
            pd2 = psA.tile([128, 256], f32, name="pd2", tag="psA")
            for m in range(2):
                ms = slice(m * 128, (m + 1) * 128)
                mmop(pd2[:, ms], W["w2d_0"][:, ms], h1d[:, 0:128],
                     start=True, stop=False)
                mmop(pd2[:, ms], W["w2d_1"][:, ms], h1d[:, 128:256],
                     start=False, stop=True)
            h2d = ap.tile([128, 256], bf16, name="h2d", tag="h2d")
            for m in range(2):
                ms = slice(m * 128, (m + 1) * 128)
                relu_evac("v", h2d[:, ms], pd2[:, ms], bias=W["b2d"][:, m:m + 1])
            pl4 = psA.tile([128, 4], f32, name="pl4", tag="psA")
            mmop(pl4, W["ones1"], W["b3d_r"], start=True, stop=False)
            mmop(pl4, h2d[:, 0:128], W["w3d_0"], start=False, stop=False)
            mmop(pl4, h2d[:, 128:256], W["w3d_1"], start=False, stop=True)
            ep4 = ap.tile([128, 4], f32, name="ep4", tag="ep4")
            s4 = sp.tile([128, 1], f32, name="s4", tag="s4")
            nc.scalar.activation(ep4, pl4, AF.Exp, accum_out=s4)
            cejunk = ap.tile([128, 4], f32, name="cejunk", tag="cejunk")
            ceacc = sp.tile([128, 1], f32, name="ceacc", tag="ceacc")
            nc.vector.tensor_tensor(cejunk, ysb, pl4, ALU.mult)
            nc.vector.tensor_reduce(ceacc, cejunk, AX.X, ALU.add)
            lns4 = sp.tile([128, 1], f32, name="lns4", tag="lns4")
            nc.scalar.activation(lns4, s4, AF.Ln)
            ceb = sp.tile([128, 1], f32, name="ceb", tag="ceb")
            nc.vector.tensor_tensor(ceb, ceacc, lns4, ALU.subtract)

            nst = nwin * WWIN
            lnse = sp.tile([128, nst], f32, name="lnse", tag="lnse")
            nc.scalar.activation(lnse, sestash[:, 0:nst], AF.Ln)
            lnsp = sp.tile([128, nst], f32, name="lnsp", tag="lnsp")
            nc.scalar.activation(lnsp, spstash[:, 0:nst], AF.Ln)
            d3 = sp.tile([128, nst], f32, name="d3", tag="d3")
            nc.vector.tensor_tensor(d3, lnsp, lnse, ALU.subtract)
            r3 = sp.tile([128, 1], f32, name="r3", tag="r3")
            nc.vector.tensor_reduce(r3, d3, AX.X, ALU.add)
            klb = sp.tile([128, 1], f32, name="klb", tag="klb")
            nc.vector.tensor_tensor(klb, klacc, r3, ALU.add)

            nc.sync.dma_start(out=out_d[0, 0:64], in_=acc1)
            nc.sync.dma_start(out=out_d[0, 64:128], in_=acc2)
            nc.sync.dma_start(out=out_d[1, :], in_=klb)
            nc.sync.dma_start(out=out_d[2, :], in_=ceb)

    nc.finalize()
    return nc


# =====================================================================
# host side
# =====================================================================

def _relu_np(a):
    return np.maximum(a, 0.0)


def _pack_consts(d):
    """name->array dict  ->  (wbf [128,CB] bf16, wf [128,CF] f32)."""
    lay, ncol_bf, ncol_f32 = _pack_layout()
    wbf = np.zeros((128, ncol_bf), BF16)
    wf = np.zeros((128, ncol_f32), F32)
    for name, arr in d.items():
        p, c, tag, off = lay[name]
        assert arr.shape == (p, c), (name, arr.shape, (p, c))
        if tag == "bf":
            wbf[0:p, off:off + c] = arr
        else:
            wf[0:p, off:off + c] = arr
    return wbf, wf


def _prep_weights(inputs):
    """Build the per-core (shared) weight arrays dict name->np array."""
    c_means = _f32(inputs["c_means"])
    p_enc = [_f32(a) for a in inputs["p_enc"]]
    p_prior = [_f32(a) for a in inputs["p_prior"]]
    p_phix = [_f32(a) for a in inputs["p_phix"]]
    p_phiz = [_f32(a) for a in inputs["p_phiz"]]
    p_cell = [_f32(a) for a in inputs["p_cell"]]
    p_pred = [_f32(a) for a in inputs["p_pred"]]
    dec = [_f32(a) for a in inputs["dec"]]
    Wih, Whh, bl, Wout, bout = dec

    d = {}

    def mlp_common(tag, W2, b2):
        d[f"w2{tag}_0"] = _bf(W2[0:128])
        d[f"w2{tag}_1"] = _bf(W2[128:256])
        d[f"b2{tag}"] = _f32(np.stack([b2[0:128], b2[128:256]], axis=1))

    # phix
    W1, b1, W2, b2, W3, b3 = p_phix
    d["w1x_aug"] = _bf(np.vstack([W1, b1[None, :]]))
    mlp_common("x", W2, b2)
    z65 = np.zeros((128, 1), F32)
    d["w3x_0"] = _bf(np.hstack([W3[0:128], z65]))
    d["w3x_1"] = _bf(np.hstack([W3[128:256], z65]))
    d["b3x_aug"] = _f32(np.concatenate([b3, [1.0]])[:, None])
    # prior
    W1, b1, W2, b2, W3, b3 = p_prior
    d["w1p_aug"] = _bf(np.vstack([W1, b1[None, :]]))
    mlp_common("p", W2, b2)
    d["w3p_0"] = _bf(W3[0:128])
    d["w3p_1"] = _bf(W3[128:256])
    d["b3p_r"] = _bf(b3[None, :])
    # enc
    W1, b1, W2, b2, W3, b3 = p_enc
    d["w1e_x"] = _bf(np.vstack([W1[0:64], b1[None, :]]))
    d["w1e_h"] = _bf(W1[64:128])
    mlp_common("e", W2, b2)
    d["w3e_0"] = _bf(W3[0:128])
    d["w3e_1"] = _bf(W3[128:256])
    d["b3e_r"] = _bf(b3[None, :])
    # phiz
    W1, b1, W2, b2, W3, b3 = p_phiz
    d["w1z_aug"] = _bf(np.vstack([W1, b1[None, :]]))
    mlp_common("z", W2, b2)
    d["w3z_0"] = _bf(np.hstack([W3[0:128], z65]))
    d["w3z_1"] = _bf(np.hstack([W3[128:256], z65]))
    d["b3z_aug"] = _f32(np.concatenate([b3, [1.0]])[:, None])
    # cell (input order: h | phx | u)
    W1, b1, W2, b2, W3, b3 = p_cell
    d["w1c_h"] = _bf(W1[0:64])
    d["w1c_x"] = _bf(np.vstack([W1[64:128], b1[None, :]]))
    d["w1c_u"] = _bf(W1[128:192])
    mlp_common("c", W2, b2)
    d["w3c_0"] = _bf(np.hstack([W3[0:128], z65]))
    d["w3c_1"] = _bf(np.hstack([W3[128:256], z65]))
    d["b3c_aug"] = _f32(np.concatenate([b3, [1.0]])[:, None])
    # pred
    W1, b1, W2, b2, W3, b3 = p_pred
    d["w1d_aug"] = _bf(np.vstack([W1, b1[None, :]]))
    mlp_common("d", W2, b2)
    d["w3d_0"] = _bf(W3[0:128])
    d["w3d_1"] = _bf(W3[128:256])
    d["b3d_r"] = _bf(b3[None, :])
    # decoder; reorder gate blocks [i f g o] -> [i f o g]
    perm = np.r_[0:128, 128:256, 384:512, 256:384]
    wih_r = Wih[:, perm].copy()
    whh_r = Whh[:, perm].copy()
    bl_r = bl[perm].copy()
    # g-gate block pre-doubled so one tanh(psum/2) covers all four gates
    wih_r[:, 384:512] *= 2.0
    whh_r[:, 384:512] *= 2.0
    bl_r[384:512] *= 2.0
    d["wih_u"] = _bf(np.vstack([wih_r[0:64], bl_r[None, :]]))
    d["wih_h"] = _bf(wih_r[64:128])
    d["whh0_u"] = _bf(whh_r[0:64])
    d["whh0_h"] = _bf(whh_r[64:128])
    d["whh_full"] = _bf(whh_r)
    d["wout_mu"] = _bf(Wout[:, 0:64])
    d["wout_lv"] = _bf(Wout[:, 64:128])
    d["nlv2"] = _f32(-0.5 * bout[64:128][:, None])
    d["bmu"] = _f32(bout[0:64][:, None])
    # misc
    d["cma"] = _bf(np.hstack([c_means, np.ones((KCL, 1), F32)]))
    d["i128"] = _bf(np.eye(128, dtype=F32))
    d["identf"] = _f32(np.eye(128, dtype=F32))
    d["ones1"] = _bf(np.ones((1, 128), F32))
    # initial carries
    z0 = c_means.mean(axis=0)
    W1, b1, W2, b2, W3, b3 = p_phiz
    u0 = _relu_np(_relu_np(_relu_np(z0 @ W1 + b1) @ W2 + b2) @ W3 + b3)
    u0a = np.concatenate([u0, [1.0]]).astype(F32)
    d["u0b"] = _bf(np.broadcast_to(u0a[:, None], (65, BL)))
    h0a = np.zeros(65, F32)
    h0a[64] = 1.0
    d["h0b"] = _bf(np.broadcast_to(h0a[:, None], (65, BL)))
    return d, bout


def _prep_x(x, nwin):
    """x (B, S, D) f32 -> per-core list of (xf [nwin,64,NT], xbf [nwin,65,NT])."""
    res = []
    for c in range(NCORES):
        xc = np.asarray(x[c * BL:(c + 1) * BL], dtype=F32)  # (BL, S, D)
        xw = xc.reshape(BL, NWIN_FULL, WWIN, D)[:, :nwin]
        # -> (nwin, D, WWIN, BL) -> (nwin, D, NT)
        xt = np.ascontiguousarray(xw.transpose(1, 3, 2, 0).reshape(nwin, D, WWIN * BL))
        ones = np.ones((nwin, 1, WWIN * BL), F32)
        xbf = np.concatenate([xt, ones], axis=1).astype(BF16)
        res.append((xt, np.ascontiguousarray(xbf)))
    return res


def kernel(**inputs):
    global LAST_RESULTS
    from concourse.bass_utils import run_bass_kernel_spmd

    nwin = int(os.environ.get("DIRVRNN_NWIN", NWIN_FULL))
    trace = os.environ.get("DIRVRNN_TRACE", "0") == "1"

    if nwin not in _CACHE:
        _CACHE[nwin] = _build(nwin)
    nc = _CACHE[nwin]

    wdict, bout = _prep_weights(inputs)
    xs = _prep_x(np.asarray(inputs["x"], dtype=F32), nwin)
    y = _f32(inputs["y"])

    in_maps = []
    for c in range(NCORES):
        d = dict(wdict)
        d["yin"] = np.ascontiguousarray(y[c * BL:(c + 1) * BL])
        wbf, wf = _pack_consts(d)
        m = {"wbf": wbf, "wf": wf}
        m["xf"], m["xbf"] = xs[c]
        in_maps.append(m)

    res = run_bass_kernel_spmd(nc, in_maps, core_ids=list(range(NCORES)),
                               trace=trace)
    LAST_RESULTS = res

    S1 = S2 = Skl = Sce = 0.0
    for r in res.results:
        o = np.asarray(r["out"], dtype=np.float64)
        S1 += o[0, 0:64].sum()
        S2 += o[0, 64:128].sum()
        Skl += o[1].sum()
        Sce += o[2].sum()

    b_lv = np.asarray(bout[64:128], dtype=np.float64)
    nsteps = nwin * WWIN
    loss = (0.5 * (S1 + S2) / B
            + 0.5 * nsteps * b_lv.sum()
            + 0.5 * LOG2PI * D * nsteps
            + Skl / B
            - Sce / B)
    return np.float32(loss)
